# revision 5
# baseline (speedup 1.0000x reference)
"""Trainium2 Bass kernel for nn_AttentionBlock (gnn_message_passing).

Key simplification: the reference softmax is over a size-1 axis, so the
attention weights are exactly 1.0 and the patch einsum collapses to a sum
over all 1024 patches.  The whole module reduces to:

  S[b,c,p1,p2] = sum_{i,j} feature[b,c,16i+p1,16j+p2]        (201 MB read)
  u[b] = S[b] . W_patch (permuted) + 1024*b_patch            [256]
  v[b] = u[b] @ W                                            [512]
  y[b] = 0.25*(v[b] @ W_out + b_out) + 0.75*token[b]
  out[b] = layernorm(y[b]) * gamma + beta

Sharding: pure data parallel, batch 64 -> 8 cores x 8 batches.  Each core
reads its 25 MB feature shard (DMA-bound), reduces it on the vector engine,
and runs the tiny matmul tail on the tensor engine.
"""

import numpy as np
from contextlib import ExitStack

import concourse.bass as bass
import concourse.tile as tile
from concourse import bacc, mybir
from concourse.bass_utils import run_bass_kernel_spmd
from concourse.masks import make_identity

F32 = mybir.dt.float32

N_CORES = 8
B = 64
C = 3
H = 512
W_IMG = 512
P = 16                      # patch size
IN_F = 256
HD = 512
BB = B // N_CORES           # 8 batches per core
EPS = 1e-5

N_ITILE = 2                 # split the 32 i-rows into 2 tiles of 16
I_PER = 32 // N_ITILE       # 16
TILE_FREE = I_PER * W_IMG   # 8192 floats = 32 KB/partition, 4 MB per tile


def _build_kernel_body(tc):
    nc = tc.nc
    feat = nc.dram_tensor("feature", [BB, C, H, W_IMG], F32, kind="ExternalInput").ap()
    tok = nc.dram_tensor("token", [BB, IN_F], F32, kind="ExternalInput").ap()
    w_pp = nc.dram_tensor("w_pp", [16, C * 16 * IN_F], F32, kind="ExternalInput").ap()
    w_mid = nc.dram_tensor("w_mid", [128, 2 * HD], F32, kind="ExternalInput").ap()
    w_out = nc.dram_tensor("w_out", [128, 4 * IN_F], F32, kind="ExternalInput").ap()
    consts = nc.dram_tensor("consts", [1, 2 * IN_F], F32, kind="ExternalInput").ap()
    gb = nc.dram_tensor("gb", [BB, 2 * IN_F], F32, kind="ExternalInput").ap()
    eye8s = nc.dram_tensor("eye8s", [BB, BB], F32, kind="ExternalInput").ap()
    out = nc.dram_tensor("out", [BB, IN_F], F32, kind="ExternalOutput").ap()

    with ExitStack() as ctx:
        mega = ctx.enter_context(tc.tile_pool(name="mega", bufs=3))
        small = ctx.enter_context(tc.tile_pool(name="small", bufs=1))
        work = ctx.enter_context(tc.tile_pool(name="work", bufs=2))
        psum = ctx.enter_context(tc.tile_pool(name="psum", bufs=2, space="PSUM"))
        psum_u = ctx.enter_context(tc.tile_pool(name="psum_u", bufs=1, space="PSUM"))
        psum_mm = ctx.enter_context(tc.tile_pool(name="psum_mm", bufs=1, space="PSUM"))

        # ---- constants / weights ----
        w_pp_t = small.tile([16, C * 16 * IN_F], F32)
        nc.sync.dma_start(w_pp_t, w_pp)
        w_mid_t = small.tile([128, 2 * HD], F32)
        nc.sync.dma_start(w_mid_t, w_mid)
        w_out_t = small.tile([128, 4 * IN_F], F32)
        nc.sync.dma_start(w_out_t, w_out)
        consts_t = small.tile([1, 2 * IN_F], F32)
        nc.sync.dma_start(consts_t, consts)
        gb_t = small.tile([BB, 2 * IN_F], F32)
        nc.sync.dma_start(gb_t, gb)
        eye8_t = small.tile([BB, BB], F32)
        nc.sync.dma_start(eye8_t, eye8s)
        tok_t = small.tile([BB, IN_F], F32)
        nc.sync.dma_start(tok_t, tok)

        ones1 = small.tile([1, BB], F32)
        nc.vector.memset(ones1, 1.0)
        eps_t = small.tile([BB, 1], F32)
        nc.vector.memset(eps_t, EPS)
        ident = small.tile([128, 128], F32)
        make_identity(nc, ident)

        # ---- stage 1: block-sum over the feature shard ----
        # per (c, h): sbuf mega tile [(b p1)=128, (i w)=8192], i in [h*16, h*16+16)
        # loaded with one 3-D DMA per batch b (partition offset b*16).
        partials = small.tile([128, C * N_ITILE * 16], F32)
        for c in range(C):
            for h in range(N_ITILE):
                mt = mega.tile([128, TILE_FREE], F32)
                for b in range(BB):
                    src = feat[b, c, h * I_PER * P:(h + 1) * I_PER * P, :].rearrange(
                        "(i p) w -> p i w", p=P
                    )
                    nc.sync.dma_start(
                        mt[b * P:(b + 1) * P, :].rearrange("p (i w) -> p i w", w=W_IMG),
                        src,
                    )
                # free index = i*512 + j*16 + q ; reduce over (i, j), keep q
                mv = mt[:].rearrange("p (i j q) -> p q i j", i=I_PER, j=32, q=16)
                nc.vector.reduce_sum(
                    partials[:, (c * N_ITILE + h) * 16:(c * N_ITILE + h + 1) * 16],
                    mv,
                    axis=mybir.AxisListType.XY,
                )

        # ---- stage 2+3 per channel: S_c -> transpose -> u matmuls ----
        u_ps = []
        for c in range(C):
            s_c = small.tile([128, 16], F32, tag=f"s{c}")
            pv = partials[:, c * N_ITILE * 16:(c + 1) * N_ITILE * 16].rearrange(
                "p (k q) -> p q k", k=N_ITILE
            )
            nc.vector.reduce_sum(s_c, pv, axis=mybir.AxisListType.X)

            st_ps = psum.tile([16, 128], F32, tag="tp")
            nc.tensor.transpose(st_ps, s_c, ident)
            st_sb = small.tile([16, 128], F32, tag=f"st{c}")
            nc.vector.tensor_copy(st_sb, st_ps)

            u_c = psum_u.tile([BB, IN_F], F32, tag=f"u{c}")
            u_ps.append(u_c)
            if c == 0:
                # bias row: u += ones.T @ (1024*b_patch)
                nc.tensor.matmul(u_c, ones1, consts_t[:, 0:IN_F], start=True, stop=False)
            stv = st_sb[:].rearrange("q (b p) -> q b p", p=P)
            for p1 in range(P):
                nc.tensor.matmul(
                    u_c,
                    stv[:, :, p1],
                    w_pp_t[:, (c * P + p1) * IN_F:(c * P + p1 + 1) * IN_F],
                    start=(c != 0 and p1 == 0),
                    stop=(p1 == P - 1),
                )

        # ---- stage 4: u = u0+u1+u2, transpose to [256, 8] ----
        u_sb = work.tile([BB, IN_F], F32)
        nc.vector.tensor_copy(u_sb, u_ps[0])
        nc.vector.tensor_add(u_sb, u_sb, u_ps[1])
        nc.vector.tensor_add(u_sb, u_sb, u_ps[2])

        uT_sb = work.tile([128, 2 * BB], F32)
        for h in range(2):
            ut_ps = psum.tile([128, BB], F32, tag="tp")
            nc.tensor.transpose(ut_ps, u_sb[:, h * 128:(h + 1) * 128], ident[0:BB, 0:BB])
            nc.vector.tensor_copy(uT_sb[:, h * BB:(h + 1) * BB], ut_ps)

        # ---- stage 5: v = u @ W ----
        v_ps = psum_mm.tile([BB, HD], F32, tag="v")
        for h in range(2):
            nc.tensor.matmul(
                v_ps,
                uT_sb[:, h * BB:(h + 1) * BB],
                w_mid_t[:, h * HD:(h + 1) * HD],
                start=(h == 0),
                stop=(h == 1),
            )
        v_sb = work.tile([BB, HD], F32)
        nc.vector.tensor_copy(v_sb, v_ps)

        vT_sb = work.tile([128, 4 * BB], F32)
        for q in range(4):
            vt_ps = psum.tile([128, BB], F32, tag="tp")
            nc.tensor.transpose(vt_ps, v_sb[:, q * 128:(q + 1) * 128], ident[0:BB, 0:BB])
            nc.vector.tensor_copy(vT_sb[:, q * BB:(q + 1) * BB], vt_ps)

        # ---- stage 6: y = 0.25*(v @ W_out + b_out) + 0.75*token ----
        y_ps = psum_mm.tile([BB, IN_F], F32, tag="y")
        nc.tensor.matmul(y_ps, eye8_t, tok_t, start=True, stop=False)        # 0.75*token
        nc.tensor.matmul(y_ps, ones1, consts_t[:, IN_F:2 * IN_F], start=False, stop=False)
        for q in range(4):
            nc.tensor.matmul(
                y_ps,
                vT_sb[:, q * BB:(q + 1) * BB],
                w_out_t[:, q * IN_F:(q + 1) * IN_F],
                start=False,
                stop=(q == 3),
            )

        # ---- stage 7: layernorm ----
        y_sb = work.tile([BB, IN_F], F32)
        nc.vector.tensor_copy(y_sb, y_ps)
        stats = work.tile([BB, 6], F32)
        nc.vector.bn_stats(stats, y_sb)
        mv2 = work.tile([BB, 2], F32)
        nc.vector.bn_aggr(mv2, stats)
        std = work.tile([BB, 1], F32)
        nc.scalar.activation(std, mv2[:, 1:2], mybir.ActivationFunctionType.Sqrt,
                             bias=eps_t, scale=1.0)
        rstd = work.tile([BB, 1], F32)
        nc.vector.reciprocal(rstd, std)
        xm = work.tile([BB, IN_F], F32)
        nc.vector.tensor_scalar(xm, y_sb, mv2[:, 0:1], rstd,
                                op0=mybir.AluOpType.subtract,
                                op1=mybir.AluOpType.mult)
        out_sb = work.tile([BB, IN_F], F32)
        nc.vector.tensor_mul(out_sb, xm, gb_t[:, 0:IN_F])
        nc.vector.tensor_add(out_sb, out_sb, gb_t[:, IN_F:2 * IN_F])
        nc.sync.dma_start(out, out_sb)


_CACHE = {}


def _get_program():
    if "nc" not in _CACHE:
        nc = bacc.Bacc("TRN2", target_bir_lowering=False, debug=False,
                       num_devices=N_CORES)
        with tile.TileContext(nc) as tc:
            _build_kernel_body(tc)
        nc.compile()
        _CACHE["nc"] = nc
    return _CACHE["nc"]


def _prep_weights(W_patch, b_patch, W, W_out, b_out, gamma, beta):
    # w_pp[p2, (c, p1, f)] = W_patch[(p1*16+p2)*3 + c, f]
    wp4 = W_patch.reshape(P, P, C, IN_F).transpose(1, 2, 0, 3)   # [p2, c, p1, f]
    w_pp = np.ascontiguousarray(wp4.reshape(P, C * P * IN_F), dtype=np.float32)
    w_mid = np.ascontiguousarray(
        np.concatenate([W[0:128, :], W[128:256, :]], axis=1), dtype=np.float32
    )
    wo = 0.25 * W_out
    w_out_t = np.ascontiguousarray(
        np.concatenate([wo[q * 128:(q + 1) * 128, :] for q in range(4)], axis=1),
        dtype=np.float32,
    )
    consts = np.ascontiguousarray(
        np.concatenate([1024.0 * b_patch, 0.25 * b_out])[None, :], dtype=np.float32
    )
    gb = np.ascontiguousarray(
        np.tile(np.concatenate([gamma, beta])[None, :], (BB, 1)), dtype=np.float32
    )
    eye8s = np.ascontiguousarray(0.75 * np.eye(BB), dtype=np.float32)
    return w_pp, w_mid, w_out_t, consts, gb, eye8s


def kernel(**inputs):
    feature = np.asarray(inputs["feature"], dtype=np.float32)
    token = np.asarray(inputs["token"], dtype=np.float32)
    w_pp, w_mid, w_out_t, consts, gb, eye8s = _prep_weights(
        np.asarray(inputs["W_patch"], dtype=np.float32),
        np.asarray(inputs["b_patch"], dtype=np.float32),
        np.asarray(inputs["W"], dtype=np.float32),
        np.asarray(inputs["W_out"], dtype=np.float32),
        np.asarray(inputs["b_out"], dtype=np.float32),
        np.asarray(inputs["gamma"], dtype=np.float32),
        np.asarray(inputs["beta"], dtype=np.float32),
    )
    nc = _get_program()
    in_maps = []
    for i in range(N_CORES):
        in_maps.append({
            "feature": np.ascontiguousarray(feature[i * BB:(i + 1) * BB]),
            "token": np.ascontiguousarray(token[i * BB:(i + 1) * BB]),
            "w_pp": w_pp,
            "w_mid": w_mid,
            "w_out": w_out_t,
            "consts": consts,
            "gb": gb,
            "eye8s": eye8s,
        })
    res = run_bass_kernel_spmd(nc, in_maps, list(range(N_CORES))).results
    return np.ascontiguousarray(
        np.concatenate([res[i]["out"] for i in range(N_CORES)], axis=0),
        dtype=np.float32,
    )


# revision 7
# speedup vs baseline: 1.4836x; 1.4836x over previous
"""Trainium2 Bass kernel for nn_AttentionBlock (gnn_message_passing).

Key simplification: the reference softmax is over a size-1 axis, so the
attention weights are exactly 1.0 and the patch einsum collapses to a sum
over all 1024 patches.  The whole module reduces to:

  S[b,c,p1,p2] = sum_{i,j} feature[b,c,16i+p1,16j+p2]        (201 MB read)
  u[b] = S[b] . W_patch (permuted) + 1024*b_patch            [256]
  v[b] = u[b] @ W                                            [512]
  y[b] = 0.25*(v[b] @ W_out + b_out) + 0.75*token[b]
  out[b] = layernorm(y[b]) * gamma + beta

Sharding: pure data parallel, batch 64 -> 8 cores x 8 batches.  Each core
reads its 25 MB feature shard (DMA-bound), reduces it on the vector engine,
and runs the tiny matmul tail on the tensor engine.
"""

import numpy as np
from contextlib import ExitStack

import concourse.bass as bass
import concourse.tile as tile
from concourse import bacc, mybir
from concourse.bass_utils import run_bass_kernel_spmd
from concourse.masks import make_identity

F32 = mybir.dt.float32

N_CORES = 8
B = 64
C = 3
H = 512
W_IMG = 512
P = 16                      # patch size
IN_F = 256
HD = 512
BB = B // N_CORES           # 8 batches per core
EPS = 1e-5

N_ITILE = 2                 # split the 32 i-rows into 2 tiles of 16
I_PER = 32 // N_ITILE       # 16
TILE_FREE = I_PER * W_IMG   # 8192 floats = 32 KB/partition, 4 MB per tile


def _build_kernel_body(tc):
    nc = tc.nc
    feat = nc.dram_tensor("feature", [BB, C, H, W_IMG], F32, kind="ExternalInput").ap()
    tok = nc.dram_tensor("token", [BB, IN_F], F32, kind="ExternalInput").ap()
    w_pp = nc.dram_tensor("w_pp", [16, C * 16 * IN_F], F32, kind="ExternalInput").ap()
    sel8 = nc.dram_tensor("sel8", [128, 16], F32, kind="ExternalInput").ap()
    w_mid = nc.dram_tensor("w_mid", [128, 2 * HD], F32, kind="ExternalInput").ap()
    w_out = nc.dram_tensor("w_out", [128, 4 * IN_F], F32, kind="ExternalInput").ap()
    consts = nc.dram_tensor("consts", [1, 2 * IN_F], F32, kind="ExternalInput").ap()
    gb = nc.dram_tensor("gb", [BB, 2 * IN_F], F32, kind="ExternalInput").ap()
    eye8s = nc.dram_tensor("eye8s", [BB, BB], F32, kind="ExternalInput").ap()
    out = nc.dram_tensor("out", [BB, IN_F], F32, kind="ExternalOutput").ap()

    with ExitStack() as ctx:
        mega = ctx.enter_context(tc.tile_pool(name="mega", bufs=8))
        small = ctx.enter_context(tc.tile_pool(name="small", bufs=1))
        work = ctx.enter_context(tc.tile_pool(name="work", bufs=2))
        psum = ctx.enter_context(tc.tile_pool(name="psum", bufs=2, space="PSUM"))
        psum_u = ctx.enter_context(tc.tile_pool(name="psum_u", bufs=1, space="PSUM"))
        psum_mm = ctx.enter_context(tc.tile_pool(name="psum_mm", bufs=1, space="PSUM"))

        # ---- constants / weights ----
        w_pp_t = small.tile([16, C * 16 * IN_F], F32)
        nc.sync.dma_start(w_pp_t, w_pp)
        w_mid_t = small.tile([128, 2 * HD], F32)
        nc.sync.dma_start(w_mid_t, w_mid)
        w_out_t = small.tile([128, 4 * IN_F], F32)
        nc.sync.dma_start(w_out_t, w_out)
        consts_t = small.tile([1, 2 * IN_F], F32)
        nc.sync.dma_start(consts_t, consts)
        gb_t = small.tile([BB, 2 * IN_F], F32)
        nc.sync.dma_start(gb_t, gb)
        eye8_t = small.tile([BB, BB], F32)
        nc.sync.dma_start(eye8_t, eye8s)
        sel8_t = small.tile([128, 16], F32)
        nc.sync.dma_start(sel8_t, sel8)
        tok_t = small.tile([BB, IN_F], F32)
        nc.sync.dma_start(tok_t, tok)

        ones1 = small.tile([1, BB], F32)
        nc.vector.memset(ones1, 1.0)
        eps_t = small.tile([BB, 1], F32)
        nc.vector.memset(eps_t, EPS)
        ident = small.tile([128, 128], F32)
        make_identity(nc, ident)

        # ---- stage 1: block-sum over the feature shard ----
        # per (b, c): one 3-D DMA, full 128 partitions, 1 MB:
        #   sbuf [r=128 rows (i8, p1), (k4, w512)];  row r_full = k*128 + r
        # DVE reduces (k, j) keeping p2; PE then collapses i8 via sel8.
        red_all = small.tile([128, BB * C * 16], F32)
        for b in range(BB):
            for c in range(C):
                mt = mega.tile([128, 4 * W_IMG], F32)
                src = feat[b, c].rearrange("(k r) w -> r k w", r=128)
                nc.sync.dma_start(
                    mt[:].rearrange("p (k w) -> p k w", w=W_IMG), src
                )
                # free index = k*512 + j*16 + q ; reduce over (k, j), keep q
                mv = mt[:].rearrange("p (k j q) -> p q k j", k=4, j=32, q=16)
                nc.vector.reduce_sum(
                    red_all[:, (b * C + c) * 16:(b * C + c + 1) * 16],
                    mv,
                    axis=mybir.AxisListType.XY,
                )

        # ---- stage 2+3 per channel: i8 partition-collapse -> u matmuls ----
        # red_all rows = (i8, p1); sel8[(i8,p1), p1'] = (p1 == p1')
        # st_c = sel8.T @ red_all_c : [p1, (b, p2)]
        redv = red_all[:].rearrange("p (b c q) -> p b c q", c=C, q=16)
        u_ps = []
        for c in range(C):
            st_ps = psum.tile([16, 128], F32, tag="tp")
            nc.tensor.matmul(st_ps, sel8_t, redv[:, :, c, :], start=True, stop=True)
            st_sb = small.tile([16, 128], F32, tag=f"st{c}")
            nc.vector.tensor_copy(st_sb, st_ps)

            u_c = psum_u.tile([BB, IN_F], F32, tag=f"u{c}")
            u_ps.append(u_c)
            if c == 0:
                # bias row: u += ones.T @ (1024*b_patch)
                nc.tensor.matmul(u_c, ones1, consts_t[:, 0:IN_F], start=True, stop=False)
            stv = st_sb[:].rearrange("k (b q) -> k b q", q=16)
            for p2 in range(P):
                nc.tensor.matmul(
                    u_c,
                    stv[:, :, p2],
                    w_pp_t[:, (c * P + p2) * IN_F:(c * P + p2 + 1) * IN_F],
                    start=(c != 0 and p2 == 0),
                    stop=(p2 == P - 1),
                )

        # ---- stage 4: u = u0+u1+u2, transpose to [256, 8] ----
        u_sb = work.tile([BB, IN_F], F32)
        nc.vector.tensor_copy(u_sb, u_ps[0])
        nc.vector.tensor_add(u_sb, u_sb, u_ps[1])
        nc.vector.tensor_add(u_sb, u_sb, u_ps[2])

        uT_sb = work.tile([128, 2 * BB], F32)
        for h in range(2):
            ut_ps = psum.tile([128, BB], F32, tag="tp")
            nc.tensor.transpose(ut_ps, u_sb[:, h * 128:(h + 1) * 128], ident[0:BB, 0:BB])
            nc.vector.tensor_copy(uT_sb[:, h * BB:(h + 1) * BB], ut_ps)

        # ---- stage 5: v = u @ W ----
        v_ps = psum_mm.tile([BB, HD], F32, tag="v")
        for h in range(2):
            nc.tensor.matmul(
                v_ps,
                uT_sb[:, h * BB:(h + 1) * BB],
                w_mid_t[:, h * HD:(h + 1) * HD],
                start=(h == 0),
                stop=(h == 1),
            )
        v_sb = work.tile([BB, HD], F32)
        nc.vector.tensor_copy(v_sb, v_ps)

        vT_sb = work.tile([128, 4 * BB], F32)
        for q in range(4):
            vt_ps = psum.tile([128, BB], F32, tag="tp")
            nc.tensor.transpose(vt_ps, v_sb[:, q * 128:(q + 1) * 128], ident[0:BB, 0:BB])
            nc.vector.tensor_copy(vT_sb[:, q * BB:(q + 1) * BB], vt_ps)

        # ---- stage 6: y = 0.25*(v @ W_out + b_out) + 0.75*token ----
        y_ps = psum_mm.tile([BB, IN_F], F32, tag="y")
        nc.tensor.matmul(y_ps, eye8_t, tok_t, start=True, stop=False)        # 0.75*token
        nc.tensor.matmul(y_ps, ones1, consts_t[:, IN_F:2 * IN_F], start=False, stop=False)
        for q in range(4):
            nc.tensor.matmul(
                y_ps,
                vT_sb[:, q * BB:(q + 1) * BB],
                w_out_t[:, q * IN_F:(q + 1) * IN_F],
                start=False,
                stop=(q == 3),
            )

        # ---- stage 7: layernorm ----
        y_sb = work.tile([BB, IN_F], F32)
        nc.vector.tensor_copy(y_sb, y_ps)
        stats = work.tile([BB, 6], F32)
        nc.vector.bn_stats(stats, y_sb)
        mv2 = work.tile([BB, 2], F32)
        nc.vector.bn_aggr(mv2, stats)
        std = work.tile([BB, 1], F32)
        nc.scalar.activation(std, mv2[:, 1:2], mybir.ActivationFunctionType.Sqrt,
                             bias=eps_t, scale=1.0)
        rstd = work.tile([BB, 1], F32)
        nc.vector.reciprocal(rstd, std)
        xm = work.tile([BB, IN_F], F32)
        nc.vector.tensor_scalar(xm, y_sb, mv2[:, 0:1], rstd,
                                op0=mybir.AluOpType.subtract,
                                op1=mybir.AluOpType.mult)
        out_sb = work.tile([BB, IN_F], F32)
        nc.vector.tensor_mul(out_sb, xm, gb_t[:, 0:IN_F])
        nc.vector.tensor_add(out_sb, out_sb, gb_t[:, IN_F:2 * IN_F])
        nc.sync.dma_start(out, out_sb)


_CACHE = {}


def _get_program():
    if "nc" not in _CACHE:
        nc = bacc.Bacc("TRN2", target_bir_lowering=False, debug=False,
                       num_devices=N_CORES)
        with tile.TileContext(nc) as tc:
            _build_kernel_body(tc)
        nc.compile()
        _CACHE["nc"] = nc
    return _CACHE["nc"]


def _prep_weights(W_patch, b_patch, W, W_out, b_out, gamma, beta):
    # w_pp[p1, (c, p2, f)] = W_patch[(p1*16+p2)*3 + c, f]
    wp4 = W_patch.reshape(P, P, C, IN_F).transpose(0, 2, 1, 3)   # [p1, c, p2, f]
    w_pp = np.ascontiguousarray(wp4.reshape(P, C * P * IN_F), dtype=np.float32)
    sel8 = np.ascontiguousarray(
        np.tile(np.eye(P, dtype=np.float32), (8, 1)))            # [128, 16]
    w_mid = np.ascontiguousarray(
        np.concatenate([W[0:128, :], W[128:256, :]], axis=1), dtype=np.float32
    )
    wo = 0.25 * W_out
    w_out_t = np.ascontiguousarray(
        np.concatenate([wo[q * 128:(q + 1) * 128, :] for q in range(4)], axis=1),
        dtype=np.float32,
    )
    consts = np.ascontiguousarray(
        np.concatenate([1024.0 * b_patch, 0.25 * b_out])[None, :], dtype=np.float32
    )
    gb = np.ascontiguousarray(
        np.tile(np.concatenate([gamma, beta])[None, :], (BB, 1)), dtype=np.float32
    )
    eye8s = np.ascontiguousarray(0.75 * np.eye(BB), dtype=np.float32)
    return w_pp, w_mid, w_out_t, consts, gb, eye8s, sel8


def kernel(**inputs):
    feature = np.asarray(inputs["feature"], dtype=np.float32)
    token = np.asarray(inputs["token"], dtype=np.float32)
    w_pp, w_mid, w_out_t, consts, gb, eys, sel8 = _prep_weights(
        np.asarray(inputs["W_patch"], dtype=np.float32),
        np.asarray(inputs["b_patch"], dtype=np.float32),
        np.asarray(inputs["W"], dtype=np.float32),
        np.asarray(inputs["W_out"], dtype=np.float32),
        np.asarray(inputs["b_out"], dtype=np.float32),
        np.asarray(inputs["gamma"], dtype=np.float32),
        np.asarray(inputs["beta"], dtype=np.float32),
    )
    nc = _get_program()
    in_maps = []
    for i in range(N_CORES):
        in_maps.append({
            "feature": np.ascontiguousarray(feature[i * BB:(i + 1) * BB]),
            "token": np.ascontiguousarray(token[i * BB:(i + 1) * BB]),
            "w_pp": w_pp,
            "w_mid": w_mid,
            "w_out": w_out_t,
            "consts": consts,
            "gb": gb,
            "eye8s": eys,
            "sel8": sel8,
        })
    res = run_bass_kernel_spmd(nc, in_maps, list(range(N_CORES))).results
    return np.ascontiguousarray(
        np.concatenate([res[i]["out"] for i in range(N_CORES)], axis=0),
        dtype=np.float32,
    )


# revision 8
# speedup vs baseline: 2.1290x; 1.4350x over previous
"""Trainium2 Bass kernel for nn_AttentionBlock (gnn_message_passing).

Key simplification: the reference softmax is over a size-1 axis, so the
attention weights are exactly 1.0 and the patch einsum collapses to a sum
over all 1024 patches.  The whole module reduces to:

  S[b,c,p1,p2] = sum_{i,j} feature[b,c,16i+p1,16j+p2]        (201 MB read)
  u[b] = S[b] . W_patch (permuted) + 1024*b_patch            [256]
  v[b] = u[b] @ W                                            [512]
  y[b] = 0.25*(v[b] @ W_out + b_out) + 0.75*token[b]
  out[b] = layernorm(y[b]) * gamma + beta

Sharding: pure data parallel, batch 64 -> 8 cores x 8 batches.  Each core
streams its 25 MB feature shard (DMA-bound), reduces (k, j) on the vector
engine, collapses the remaining cross-partition i-dim with a 0/1 matmul,
and runs the small matmul tail in float32r (single-pass fp32, ~1e-4).
"""

import numpy as np
from contextlib import ExitStack

import concourse.bass as bass
import concourse.tile as tile
from concourse import bacc, mybir
from concourse.bass_utils import run_bass_kernel_spmd

F32 = mybir.dt.float32
F32R = mybir.dt.float32r

N_CORES = 8
B = 64
C = 3
H = 512
W_IMG = 512
P = 16                      # patch size
IN_F = 256
HD = 512
BB = B // N_CORES           # 8 batches per core
EPS = 1e-5


def _build_kernel_body(tc):
    nc = tc.nc
    feat = nc.dram_tensor("feature", [BB, C, H, W_IMG], F32, kind="ExternalInput").ap()
    tok_adj = nc.dram_tensor("tok_adj", [BB, IN_F], F32, kind="ExternalInput").ap()
    bias_u = nc.dram_tensor("bias_u", [BB, IN_F], F32, kind="ExternalInput").ap()
    gb = nc.dram_tensor("gb", [BB, 2 * IN_F], F32, kind="ExternalInput").ap()
    sel8 = nc.dram_tensor("sel8", [128, 16], F32, kind="ExternalInput").ap()
    ident8 = nc.dram_tensor("ident8", [BB, BB], F32, kind="ExternalInput").ap()
    w_pp = nc.dram_tensor("w_pp", [16, C * 16 * IN_F], F32R, kind="ExternalInput").ap()
    w_mid = nc.dram_tensor("w_mid", [128, 2 * HD], F32R, kind="ExternalInput").ap()
    w_out = nc.dram_tensor("w_out", [128, 4 * IN_F], F32R, kind="ExternalInput").ap()
    out = nc.dram_tensor("out", [BB, IN_F], F32, kind="ExternalOutput").ap()

    with ExitStack() as ctx:
        mega = ctx.enter_context(tc.tile_pool(name="mega", bufs=8))
        small = ctx.enter_context(tc.tile_pool(name="small", bufs=1))
        work = ctx.enter_context(tc.tile_pool(name="work", bufs=2))
        psum = ctx.enter_context(tc.tile_pool(name="psum", bufs=2, space="PSUM"))
        psum_u = ctx.enter_context(tc.tile_pool(name="psum_u", bufs=1, space="PSUM"))
        psum_mm = ctx.enter_context(tc.tile_pool(name="psum_mm", bufs=1, space="PSUM"))

        # ---- constants / weights ----
        w_pp_t = small.tile([16, C * 16 * IN_F], F32R)
        nc.sync.dma_start(w_pp_t, w_pp)
        w_mid_t = small.tile([128, 2 * HD], F32R)
        nc.sync.dma_start(w_mid_t, w_mid)
        w_out_t = small.tile([128, 4 * IN_F], F32R)
        nc.sync.dma_start(w_out_t, w_out)
        gb_t = small.tile([BB, 2 * IN_F], F32)
        nc.sync.dma_start(gb_t, gb)
        tok_t = small.tile([BB, IN_F], F32)
        nc.sync.dma_start(tok_t, tok_adj)
        bias_u_t = small.tile([BB, IN_F], F32)
        nc.sync.dma_start(bias_u_t, bias_u)
        sel8_t = small.tile([128, 16], F32)
        nc.sync.dma_start(sel8_t, sel8)
        ident8_t = small.tile([BB, BB], F32)
        nc.sync.dma_start(ident8_t, ident8)
        eps_t = small.tile([BB, 1], F32)
        nc.vector.memset(eps_t, EPS)

        # ---- stage 1: block-sum over the feature shard ----
        # per (b, c): one 3-D DMA, full 128 partitions, 1 MB:
        #   sbuf [r=128 rows (i8, p1), (k4, w512)];  row r_full = k*128 + r
        # DVE reduces (k, j) keeping p2; PE then collapses i8 via sel8.
        red_all = small.tile([128, BB * C * 16], F32)
        dma_engines = [nc.sync, nc.scalar]
        for b in range(BB):
            for c in range(C):
                mt = mega.tile([128, 4 * W_IMG], F32)
                src = feat[b, c].rearrange("(k r) w -> r k w", r=128)
                dma_engines[(b * C + c) % 2].dma_start(
                    mt[:].rearrange("p (k w) -> p k w", w=W_IMG), src
                )
                # free index = k*512 + j*16 + q ; reduce over (k, j), keep q
                mv = mt[:].rearrange("p (k j q) -> p q k j", k=4, j=32, q=16)
                nc.vector.reduce_sum(
                    red_all[:, (b * C + c) * 16:(b * C + c + 1) * 16],
                    mv,
                    axis=mybir.AxisListType.XY,
                )

        # ---- stage 2+3 per channel: i8 partition-collapse -> u matmuls ----
        # red_all rows = (i8, p1); sel8[(i8,p1), p1'] = (p1 == p1')
        # st_c = sel8.T @ red_all_c : [p1, (b, p2)]
        redv = red_all[:].rearrange("p (b c q) -> p b c q", c=C, q=16)
        u_ps = psum_u.tile([BB, IN_F], F32, tag="u")
        for c in range(C):
            st_ps = psum.tile([16, 128], F32, tag="tp")
            nc.tensor.matmul(st_ps, sel8_t, redv[:, :, c, :], start=True, stop=True)
            st_sb = small.tile([16, 128], F32R, tag=f"st{c}")
            nc.vector.tensor_copy(st_sb, st_ps)

            stv = st_sb[:].rearrange("k (b q) -> k b q", q=16)
            for p2 in range(P):
                nc.tensor.matmul(
                    u_ps,
                    stv[:, :, p2],
                    w_pp_t[:, (c * P + p2) * IN_F:(c * P + p2 + 1) * IN_F],
                    start=(c == 0 and p2 == 0),
                    stop=(c == C - 1 and p2 == P - 1),
                )

        # ---- stage 4: u = u_ps + 1024*b_patch, transpose to [256, 8] ----
        u_sb = work.tile([BB, IN_F], F32)
        nc.vector.tensor_add(u_sb, u_ps, bias_u_t)

        uT_sb = work.tile([128, 2 * BB], F32R)
        for h in range(2):
            ut_ps = psum.tile([128, BB], F32, tag="tp2")
            nc.tensor.transpose(ut_ps, u_sb[:, h * 128:(h + 1) * 128], ident8_t)
            nc.vector.tensor_copy(uT_sb[:, h * BB:(h + 1) * BB], ut_ps)

        # ---- stage 5: v = u @ W ----
        v_ps = psum_mm.tile([BB, HD], F32, tag="v")
        for h in range(2):
            nc.tensor.matmul(
                v_ps,
                uT_sb[:, h * BB:(h + 1) * BB],
                w_mid_t[:, h * HD:(h + 1) * HD],
                start=(h == 0),
                stop=(h == 1),
            )
        v_sb = work.tile([BB, HD], F32)
        nc.vector.tensor_copy(v_sb, v_ps)

        vT_sb = work.tile([128, 4 * BB], F32R)
        for q in range(4):
            vt_ps = psum.tile([128, BB], F32, tag="tp2")
            nc.tensor.transpose(vt_ps, v_sb[:, q * 128:(q + 1) * 128], ident8_t)
            nc.vector.tensor_copy(vT_sb[:, q * BB:(q + 1) * BB], vt_ps)

        # ---- stage 6: y = 0.25*v@W_out + (0.25*b_out + 0.75*token) ----
        y_ps = psum_mm.tile([BB, IN_F], F32, tag="y")
        for q in range(4):
            nc.tensor.matmul(
                y_ps,
                vT_sb[:, q * BB:(q + 1) * BB],
                w_out_t[:, q * IN_F:(q + 1) * IN_F],
                start=(q == 0),
                stop=(q == 3),
            )
        y_sb = work.tile([BB, IN_F], F32)
        nc.vector.tensor_add(y_sb, y_ps, tok_t)

        # ---- stage 7: layernorm ----
        stats = work.tile([BB, 6], F32)
        nc.vector.bn_stats(stats, y_sb)
        mv2 = work.tile([BB, 2], F32)
        nc.vector.bn_aggr(mv2, stats)
        std = work.tile([BB, 1], F32)
        nc.scalar.activation(std, mv2[:, 1:2], mybir.ActivationFunctionType.Sqrt,
                             bias=eps_t, scale=1.0)
        rstd = work.tile([BB, 1], F32)
        nc.vector.reciprocal(rstd, std)
        xm = work.tile([BB, IN_F], F32)
        nc.vector.tensor_scalar(xm, y_sb, mv2[:, 0:1], rstd,
                                op0=mybir.AluOpType.subtract,
                                op1=mybir.AluOpType.mult)
        out_sb = work.tile([BB, IN_F], F32)
        nc.vector.tensor_mul(out_sb, xm, gb_t[:, 0:IN_F])
        nc.vector.tensor_add(out_sb, out_sb, gb_t[:, IN_F:2 * IN_F])
        nc.sync.dma_start(out, out_sb)


_CACHE = {}


def _get_program():
    if "nc" not in _CACHE:
        nc = bacc.Bacc("TRN2", target_bir_lowering=False, debug=False,
                       num_devices=N_CORES)
        with tile.TileContext(nc) as tc:
            _build_kernel_body(tc)
        nc.compile()
        _CACHE["nc"] = nc
    return _CACHE["nc"]


def _prep_weights(W_patch, b_patch, W, W_out, b_out, gamma, beta):
    # w_pp[p1, (c, p2, f)] = W_patch[(p1*16+p2)*3 + c, f]
    wp4 = W_patch.reshape(P, P, C, IN_F).transpose(0, 2, 1, 3)   # [p1, c, p2, f]
    w_pp = np.ascontiguousarray(wp4.reshape(P, C * P * IN_F), dtype=np.float32)
    sel8 = np.ascontiguousarray(
        np.tile(np.eye(P, dtype=np.float32), (8, 1)))            # [128, 16]
    w_mid = np.ascontiguousarray(
        np.concatenate([W[0:128, :], W[128:256, :]], axis=1), dtype=np.float32
    )
    wo = 0.25 * W_out
    w_out_t = np.ascontiguousarray(
        np.concatenate([wo[q * 128:(q + 1) * 128, :] for q in range(4)], axis=1),
        dtype=np.float32,
    )
    bias_u = np.ascontiguousarray(
        np.tile((1024.0 * b_patch)[None, :], (BB, 1)), dtype=np.float32
    )
    gb = np.ascontiguousarray(
        np.tile(np.concatenate([gamma, beta])[None, :], (BB, 1)), dtype=np.float32
    )
    ident8 = np.ascontiguousarray(np.eye(BB), dtype=np.float32)
    return w_pp, w_mid, w_out_t, bias_u, gb, ident8, sel8


def kernel(**inputs):
    feature = np.asarray(inputs["feature"], dtype=np.float32)
    token = np.asarray(inputs["token"], dtype=np.float32)
    b_out = np.asarray(inputs["b_out"], dtype=np.float32)
    w_pp, w_mid, w_out_t, bias_u, gb, ident8, sel8 = _prep_weights(
        np.asarray(inputs["W_patch"], dtype=np.float32),
        np.asarray(inputs["b_patch"], dtype=np.float32),
        np.asarray(inputs["W"], dtype=np.float32),
        np.asarray(inputs["W_out"], dtype=np.float32),
        b_out,
        np.asarray(inputs["gamma"], dtype=np.float32),
        np.asarray(inputs["beta"], dtype=np.float32),
    )
    tok_adj = (0.75 * token + 0.25 * b_out[None, :]).astype(np.float32)
    nc = _get_program()
    in_maps = []
    for i in range(N_CORES):
        in_maps.append({
            "feature": np.ascontiguousarray(feature[i * BB:(i + 1) * BB]),
            "tok_adj": np.ascontiguousarray(tok_adj[i * BB:(i + 1) * BB]),
            "bias_u": bias_u,
            "gb": gb,
            "sel8": sel8,
            "ident8": ident8,
            "w_pp": w_pp,
            "w_mid": w_mid,
            "w_out": w_out_t,
        })
    res = run_bass_kernel_spmd(nc, in_maps, list(range(N_CORES))).results
    return np.ascontiguousarray(
        np.concatenate([res[i]["out"] for i in range(N_CORES)], axis=0),
        dtype=np.float32,
    )


# revision 9
# speedup vs baseline: 2.3939x; 1.1244x over previous
"""Trainium2 Bass kernel for nn_AttentionBlock (gnn_message_passing).

Key simplification: the reference softmax is over a size-1 axis, so the
attention weights are exactly 1.0 and the patch einsum collapses to a sum
over all 1024 patches.  The whole module reduces to:

  S[b,c,p1,p2] = sum_{i,j} feature[b,c,16i+p1,16j+p2]        (201 MB read)
  u[b] = S[b] . W_patch (permuted) + 1024*b_patch            [256]
  v[b] = u[b] @ W                                            [512]
  y[b] = 0.25*(v[b] @ W_out + b_out) + 0.75*token[b]
  out[b] = layernorm(y[b]) * gamma + beta

Sharding: pure data parallel, batch 64 -> 8 cores x 8 batches.  Each core
streams its 25 MB feature shard (DMA-bound), reduces (k, j) on the vector
engine, collapses the remaining cross-partition i-dim with a 0/1 matmul,
and runs the small matmul tail in float32r (single-pass fp32, ~1e-4).
"""

import numpy as np
from contextlib import ExitStack

import concourse.bass as bass
import concourse.tile as tile
from concourse import bacc, mybir
from concourse.bass_utils import run_bass_kernel_spmd

F32 = mybir.dt.float32
F32R = mybir.dt.float32r

N_CORES = 8
B = 64
C = 3
H = 512
W_IMG = 512
P = 16                      # patch size
IN_F = 256
HD = 512
BB = B // N_CORES           # 8 batches per core
EPS = 1e-5


def _build_kernel_body(tc):
    nc = tc.nc
    feat = nc.dram_tensor("feature", [BB, C, H, W_IMG], F32, kind="ExternalInput").ap()
    tok_adj = nc.dram_tensor("tok_adj", [BB, IN_F], F32, kind="ExternalInput").ap()
    bias_u = nc.dram_tensor("bias_u", [BB, IN_F], F32, kind="ExternalInput").ap()
    gb = nc.dram_tensor("gb", [BB, 2 * IN_F], F32, kind="ExternalInput").ap()
    sel8 = nc.dram_tensor("sel8", [128, 16], F32, kind="ExternalInput").ap()
    ident8 = nc.dram_tensor("ident8", [BB, BB], F32, kind="ExternalInput").ap()
    w_pp = nc.dram_tensor("w_pp", [16, C * 16 * IN_F], F32R, kind="ExternalInput").ap()
    w_mid = nc.dram_tensor("w_mid", [128, 2 * HD], F32R, kind="ExternalInput").ap()
    w_out = nc.dram_tensor("w_out", [128, 4 * IN_F], F32R, kind="ExternalInput").ap()
    out = nc.dram_tensor("out", [BB, IN_F], F32, kind="ExternalOutput").ap()

    with ExitStack() as ctx:
        mega = ctx.enter_context(tc.tile_pool(name="mega", bufs=8))
        small = ctx.enter_context(tc.tile_pool(name="small", bufs=1))
        work = ctx.enter_context(tc.tile_pool(name="work", bufs=2))
        psum = ctx.enter_context(tc.tile_pool(name="psum", bufs=2, space="PSUM"))
        psum_u = ctx.enter_context(tc.tile_pool(name="psum_u", bufs=1, space="PSUM"))
        psum_mm = ctx.enter_context(tc.tile_pool(name="psum_mm", bufs=1, space="PSUM"))

        # ---- constants / weights ----
        w_pp_t = small.tile([16, C * 16 * IN_F], F32R)
        nc.sync.dma_start(w_pp_t, w_pp)
        w_mid_t = small.tile([128, 2 * HD], F32R)
        nc.sync.dma_start(w_mid_t, w_mid)
        w_out_t = small.tile([128, 4 * IN_F], F32R)
        nc.sync.dma_start(w_out_t, w_out)
        gb_t = small.tile([BB, 2 * IN_F], F32)
        nc.sync.dma_start(gb_t, gb)
        tok_t = small.tile([BB, IN_F], F32)
        nc.sync.dma_start(tok_t, tok_adj)
        bias_u_t = small.tile([BB, IN_F], F32)
        nc.sync.dma_start(bias_u_t, bias_u)
        sel8_t = small.tile([128, 16], F32)
        nc.sync.dma_start(sel8_t, sel8)
        ident8_t = small.tile([BB, BB], F32)
        nc.sync.dma_start(ident8_t, ident8)
        eps_t = small.tile([BB, 1], F32)
        nc.vector.memset(eps_t, EPS)

        # ---- stage 1 + 2 + 3, channel-major so the tail pipelines ----
        # per (b, c): one 3-D DMA, full 128 partitions, 1 MB:
        #   sbuf [r=128 rows (i8, p1), (k4, w512)];  row r_full = k*128 + r
        # (k, j)-collapse: two contiguous tensor-adds (first one on GpSimd for
        # half the pairs to offload DVE), then a 3-D reduce keeping p2.
        # PE collapses i8 via sel8 and runs the per-channel u matmuls.
        red_all = small.tile([128, BB * C * 16], F32)
        redv = red_all[:].rearrange("p (b c q) -> p b c q", c=C, q=16)
        u_ps = psum_u.tile([BB, IN_F], F32, tag="u")
        dma_engines = [nc.sync, nc.scalar]
        for c in range(C):
            for b in range(BB):
                idx = c * BB + b
                mt = mega.tile([128, 4 * W_IMG], F32)
                src = feat[b, c].rearrange("(k r) w -> r k w", r=128)
                dma_engines[idx % 2].dma_start(
                    mt[:].rearrange("p (k w) -> p k w", w=W_IMG), src
                )
                # free index = k*512 + j*16 + q
                h1 = work.tile([128, 2 * W_IMG], F32, tag="h1", bufs=4)
                eng = nc.gpsimd if idx % 2 == 0 else nc.vector
                eng.tensor_add(h1, mt[:, 0:2 * W_IMG], mt[:, 2 * W_IMG:4 * W_IMG])
                h2 = work.tile([128, W_IMG], F32, tag="h2", bufs=4)
                nc.vector.tensor_add(h2, h1[:, 0:W_IMG], h1[:, W_IMG:2 * W_IMG])
                # h2 free index = j*16 + q (k folded); reduce j, keep q
                mv = h2[:].rearrange("p (j q) -> p q j", q=16)
                nc.vector.reduce_sum(
                    red_all[:, (b * C + c) * 16:(b * C + c + 1) * 16],
                    mv,
                    axis=mybir.AxisListType.X,
                )

            # ---- per channel: i8 partition-collapse -> u matmuls ----
            # red_all rows = (i8, p1); sel8[(i8,p1), p1'] = (p1 == p1')
            # st_c = sel8.T @ red_all_c : [p1, (b, p2)]
            st_ps = psum.tile([16, 128], F32, tag="tp")
            nc.tensor.matmul(st_ps, sel8_t, redv[:, :, c, :], start=True, stop=True)
            st_sb = small.tile([16, 128], F32R, tag=f"st{c}")
            nc.vector.tensor_copy(st_sb, st_ps)

            stv = st_sb[:].rearrange("k (b q) -> k b q", q=16)
            for p2 in range(P):
                nc.tensor.matmul(
                    u_ps,
                    stv[:, :, p2],
                    w_pp_t[:, (c * P + p2) * IN_F:(c * P + p2 + 1) * IN_F],
                    start=(c == 0 and p2 == 0),
                    stop=(c == C - 1 and p2 == P - 1),
                )

        # ---- stage 4: u = u_ps + 1024*b_patch, transpose to [256, 8] ----
        u_sb = work.tile([BB, IN_F], F32)
        nc.vector.tensor_add(u_sb, u_ps, bias_u_t)

        uT_sb = work.tile([128, 2 * BB], F32R)
        for h in range(2):
            ut_ps = psum.tile([128, BB], F32, tag="tp2")
            nc.tensor.transpose(ut_ps, u_sb[:, h * 128:(h + 1) * 128], ident8_t)
            nc.vector.tensor_copy(uT_sb[:, h * BB:(h + 1) * BB], ut_ps)

        # ---- stage 5: v = u @ W ----
        v_ps = psum_mm.tile([BB, HD], F32, tag="v")
        for h in range(2):
            nc.tensor.matmul(
                v_ps,
                uT_sb[:, h * BB:(h + 1) * BB],
                w_mid_t[:, h * HD:(h + 1) * HD],
                start=(h == 0),
                stop=(h == 1),
            )
        v_sb = work.tile([BB, HD], F32)
        nc.vector.tensor_copy(v_sb, v_ps)

        vT_sb = work.tile([128, 4 * BB], F32R)
        for q in range(4):
            vt_ps = psum.tile([128, BB], F32, tag="tp2")
            nc.tensor.transpose(vt_ps, v_sb[:, q * 128:(q + 1) * 128], ident8_t)
            nc.vector.tensor_copy(vT_sb[:, q * BB:(q + 1) * BB], vt_ps)

        # ---- stage 6: y = 0.25*v@W_out + (0.25*b_out + 0.75*token) ----
        y_ps = psum_mm.tile([BB, IN_F], F32, tag="y")
        for q in range(4):
            nc.tensor.matmul(
                y_ps,
                vT_sb[:, q * BB:(q + 1) * BB],
                w_out_t[:, q * IN_F:(q + 1) * IN_F],
                start=(q == 0),
                stop=(q == 3),
            )
        y_sb = work.tile([BB, IN_F], F32)
        nc.vector.tensor_add(y_sb, y_ps, tok_t)

        # ---- stage 7: layernorm ----
        stats = work.tile([BB, 6], F32)
        nc.vector.bn_stats(stats, y_sb)
        mv2 = work.tile([BB, 2], F32)
        nc.vector.bn_aggr(mv2, stats)
        std = work.tile([BB, 1], F32)
        nc.scalar.activation(std, mv2[:, 1:2], mybir.ActivationFunctionType.Sqrt,
                             bias=eps_t, scale=1.0)
        rstd = work.tile([BB, 1], F32)
        nc.vector.reciprocal(rstd, std)
        xm = work.tile([BB, IN_F], F32)
        nc.vector.tensor_scalar(xm, y_sb, mv2[:, 0:1], rstd,
                                op0=mybir.AluOpType.subtract,
                                op1=mybir.AluOpType.mult)
        out_sb = work.tile([BB, IN_F], F32)
        nc.vector.tensor_mul(out_sb, xm, gb_t[:, 0:IN_F])
        nc.vector.tensor_add(out_sb, out_sb, gb_t[:, IN_F:2 * IN_F])
        nc.sync.dma_start(out, out_sb)


_CACHE = {}


def _get_program():
    if "nc" not in _CACHE:
        nc = bacc.Bacc("TRN2", target_bir_lowering=False, debug=False,
                       num_devices=N_CORES)
        with tile.TileContext(nc) as tc:
            _build_kernel_body(tc)
        nc.compile()
        _CACHE["nc"] = nc
    return _CACHE["nc"]


def _prep_weights(W_patch, b_patch, W, W_out, b_out, gamma, beta):
    # w_pp[p1, (c, p2, f)] = W_patch[(p1*16+p2)*3 + c, f]
    wp4 = W_patch.reshape(P, P, C, IN_F).transpose(0, 2, 1, 3)   # [p1, c, p2, f]
    w_pp = np.ascontiguousarray(wp4.reshape(P, C * P * IN_F), dtype=np.float32)
    sel8 = np.ascontiguousarray(
        np.tile(np.eye(P, dtype=np.float32), (8, 1)))            # [128, 16]
    w_mid = np.ascontiguousarray(
        np.concatenate([W[0:128, :], W[128:256, :]], axis=1), dtype=np.float32
    )
    wo = 0.25 * W_out
    w_out_t = np.ascontiguousarray(
        np.concatenate([wo[q * 128:(q + 1) * 128, :] for q in range(4)], axis=1),
        dtype=np.float32,
    )
    bias_u = np.ascontiguousarray(
        np.tile((1024.0 * b_patch)[None, :], (BB, 1)), dtype=np.float32
    )
    gb = np.ascontiguousarray(
        np.tile(np.concatenate([gamma, beta])[None, :], (BB, 1)), dtype=np.float32
    )
    ident8 = np.ascontiguousarray(np.eye(BB), dtype=np.float32)
    return w_pp, w_mid, w_out_t, bias_u, gb, ident8, sel8


def kernel(**inputs):
    feature = np.asarray(inputs["feature"], dtype=np.float32)
    token = np.asarray(inputs["token"], dtype=np.float32)
    b_out = np.asarray(inputs["b_out"], dtype=np.float32)
    w_pp, w_mid, w_out_t, bias_u, gb, ident8, sel8 = _prep_weights(
        np.asarray(inputs["W_patch"], dtype=np.float32),
        np.asarray(inputs["b_patch"], dtype=np.float32),
        np.asarray(inputs["W"], dtype=np.float32),
        np.asarray(inputs["W_out"], dtype=np.float32),
        b_out,
        np.asarray(inputs["gamma"], dtype=np.float32),
        np.asarray(inputs["beta"], dtype=np.float32),
    )
    tok_adj = (0.75 * token + 0.25 * b_out[None, :]).astype(np.float32)
    nc = _get_program()
    in_maps = []
    for i in range(N_CORES):
        in_maps.append({
            "feature": np.ascontiguousarray(feature[i * BB:(i + 1) * BB]),
            "tok_adj": np.ascontiguousarray(tok_adj[i * BB:(i + 1) * BB]),
            "bias_u": bias_u,
            "gb": gb,
            "sel8": sel8,
            "ident8": ident8,
            "w_pp": w_pp,
            "w_mid": w_mid,
            "w_out": w_out_t,
        })
    res = run_bass_kernel_spmd(nc, in_maps, list(range(N_CORES))).results
    return np.ascontiguousarray(
        np.concatenate([res[i]["out"] for i in range(N_CORES)], axis=0),
        dtype=np.float32,
    )
